# revision 8
# baseline (speedup 1.0000x reference)
"""CIN (Compressed Interaction Network) forward pass on 8 Trainium2 cores.

Reference computation (per sample b, per embedding dim d — fully pointwise
in (b, d)):
    x0 = inputs[b, :, d]                    # [40]
    h  = x0
    per layer i (W_i maps Fi*40 -> 256):
        z  = outer(h, x0).flatten()         # [Fi * 40], index f*40+g
        y  = relu(z @ W_i + b_i)            # [256]
        outputs_i = y[:128] (i<3) else y    # pooled
        h  = y[128:]                        # fields fed forward
    out[b] = sum_d concat(outputs)          # [512]

Sharding: data-parallel over batch (64 samples/core), weights replicated.

Per-core kernel strategy (v2 — fp8 DoubleRow):
  - Blocked over bd = (b*32 + d) in chunks of 512 columns; column-major
    layouts (h^T [fields, bd]).
  - Layer 1 exploits z-symmetry (z[f,g]=z[g,f]): contraction shrinks from
    1600 to 820 rows (padded to 896 = 7x128) with symmetrized weights
    W1s[(f,g)] = W1[f,g]+W1[g,f].  z1 tiles are pure functions of the input
    and are precomputed on the host (fp32 products, single fp16 rounding)
    and streamed from HBM straight to the tensor engine.
  - Layers 2/3 run the tensor engine in fp8-e4m3 DoubleRow mode: weights are
    host-prescaled by 2^11 (undone by the activation scale) and packed as
    [f, pair, 2, half, 128]; each matmul contracts 256 rows (128 fields x 2
    g-values) at 0.5 cycles/output-column — ~2x the bf16 rate.
  - z tiles for L2/L3 are built on the Vector engine in fp16 2x-packed mode
    (groups of 4 g-values per op: z16[f, j, :] = h[f, :] * x0b[f, g0+j, :]),
    then CONVERTED to fp8 on the Scalar (ACT) and GpSimd (Pool) engines,
    alternating, so no single engine becomes the bottleneck.  The DVE never
    writes fp8 directly (1-byte operands drop it to 1x throughput).
  - The x0 row needed on all 128 partitions is materialized once per chunk
    by DMAs with a stride-0 partition source (quartered, alternating HWDGE
    rings so consumers start on the first quarter).
  - Scalar engine applies scale+bias+ReLU straight out of PSUM -> fp16.
  - GpSimd engine reduces y-halves over d (groups of 32) into the output
    accumulators; final DMA writes [512 fields, 64 batch] per core.
  - Layer order is software-pipelined L1(0), [L2(c), L1(c+1), L3(c)] so the
    tensor engine never waits on the h hand-off between layers.
"""

import numpy as np

import concourse.bass as bass
import concourse.tile as tile
from concourse import bacc, mybir
from concourse.bass import ds

F32 = mybir.dt.float32
F16 = mybir.dt.float16
F8 = mybir.dt.float8e4

B, F0, D = 512, 40, 32
N_CORES = 8
B_CORE = B // N_CORES            # 64
BD = B_CORE * D                  # 2048
CHUNK = 512
N_CHUNKS = BD // CHUNK           # 4
B_CHUNK = CHUNK // D             # 16 batch rows per chunk
FI = 128                         # h fields for layers 2/3
NOUT = 256
L1_T = 7                         # 7 k-tiles x 128 = 896 >= 820 sym rows
NSYM = F0 * (F0 + 1) // 2        # 820
NPAIR = F0 // 2                  # 20 DoubleRow pairs per layer
GG = 8                           # g-values per DVE z-build op (4 pairs)
NGRP = F0 // GG                  # 5 groups per layer-chunk
WSHIFT = 11                      # W2/W3 prescale exponent
WSCALE = float(2.0 ** WSHIFT)
ISCALE = float(2.0 ** -WSHIFT)
RELU = mybir.ActivationFunctionType.Relu
COPY = mybir.ActivationFunctionType.Copy
DR = mybir.MatmulPerfMode.DoubleRow

_BUILD_CACHE = {}


def _build(reps=1, trace_sim=False, psum_bufs=8, z8_bufs=6, z16_bufs=2,
           x0b_split=4, cvt_pool_share=2):
    """Build + schedule + bacc-compile the per-core program.

    cvt_pool_share: out of every 4 convert groups, how many go to the Pool
    engine (rest go to ACT).
    """
    nc = bacc.Bacc("TRN2", target_bir_lowering=False, debug=False,
                   num_devices=N_CORES)

    x0t = nc.declare_dram_parameter("x0t", [N_CHUNKS, F0, CHUNK], F16, isOutput=False)
    z1 = nc.declare_dram_parameter("z1", [N_CHUNKS, 128, L1_T, CHUNK], F16, isOutput=False)
    w1s = nc.declare_dram_parameter("w1s", [128, L1_T, NOUT], F16, isOutput=False)
    w2 = nc.declare_dram_parameter("w2", [FI, NPAIR, 2, 2, 128], F8, isOutput=False)
    w3 = nc.declare_dram_parameter("w3", [FI, NPAIR, 2, 2, 128], F8, isOutput=False)
    b1 = nc.declare_dram_parameter("b1", [NOUT], F32, isOutput=False)
    b2 = nc.declare_dram_parameter("b2", [NOUT], F32, isOutput=False)
    b3 = nc.declare_dram_parameter("b3", [NOUT], F32, isOutput=False)
    out = nc.declare_dram_parameter("out", [4 * FI, B_CORE], F32, isOutput=True)

    with tile.TileContext(nc, trace_sim=trace_sim) as tc:
        import contextlib
        with contextlib.ExitStack() as ctx:
            wpool = ctx.enter_context(tc.tile_pool(name="w", bufs=1))
            opool = ctx.enter_context(tc.tile_pool(name="o", bufs=1))
            x0bpool = ctx.enter_context(tc.tile_pool(name="x0b", bufs=2))
            l1pool = ctx.enter_context(tc.tile_pool(name="l1", bufs=2))
            z16pool = ctx.enter_context(tc.tile_pool(name="z16", bufs=z16_bufs))
            z8pool = ctx.enter_context(tc.tile_pool(name="z8", bufs=z8_bufs))
            hpool = ctx.enter_context(tc.tile_pool(name="h", bufs=3))
            ypool = ctx.enter_context(tc.tile_pool(name="y", bufs=4))
            pspool = ctx.enter_context(tc.tile_pool(name="ps", bufs=psum_bufs, space="PSUM"))

            # ---- resident constants ----
            w1s_sb = wpool.tile([128, L1_T, NOUT], F16, tag="w1s")
            nc.scalar.dma_start(out=w1s_sb[:], in_=w1s[:])
            # In the real (reps=1) build, w2/w3 DMAs are emitted inside
            # emit_body AFTER chunk 0's critical loads so startup isn't
            # serialized behind weights layer 1 doesn't need.
            w2_sb = wpool.tile([FI, NPAIR, 2, 2, 128], F8, tag="w2")
            w3_sb = wpool.tile([FI, NPAIR, 2, 2, 128], F8, tag="w3")
            if reps != 1:
                nc.sync.dma_start(out=w2_sb[:], in_=w2[:])
                nc.scalar.dma_start(out=w3_sb[:], in_=w3[:])
            bias = {}
            for nm, t in (("b1", b1), ("b2", b2), ("b3", b3)):
                for half in range(2):
                    bt = wpool.tile([FI, 1], F32, tag=f"{nm}_{half}", name=f"{nm}_{half}")
                    nc.scalar.dma_start(out=bt[:], in_=t[ds(half * FI, FI)].unsqueeze(1))
                    bias[(nm, half)] = bt
            # output accumulators [fields(128) x batch], one per field block
            oacc = [opool.tile([FI, B_CORE], F32, tag=f"oacc{k}", name=f"oacc{k}")
                    for k in range(4)]

            pending_acts = []     # deferred (psum, bias_ap, scale, oidx, c)
            pending_reduces = []  # deferred (y, oidx, c)
            h_tiles = {}

            def act_pool_half(ps_half, bias_ap, sc, oidx, c):
                """Deferred relu+bias on ACT -> fp16; reduce deferred again."""
                pending_acts.append((ps_half, bias_ap, sc, oidx, c))

            def flush_y_acts():
                while pending_acts:
                    ps_half, bias_ap, sc, oidx, c = pending_acts.pop(0)
                    y = ypool.tile([FI, CHUNK], F16, tag="y", name=f"y_{oidx}_{c}")
                    nc.scalar.activation(y[:], ps_half[:], RELU, bias=bias_ap,
                                         scale=sc)
                    pending_reduces.append((y, oidx, c))

            def flush_reduces():
                # d-sum reduces on the DVE; emitted a chunk later than their
                # y so the in-order DVE stream never stalls waiting on ACT.
                while pending_reduces:
                    y, oidx, c = pending_reduces.pop(0)
                    nc.vector.tensor_reduce(
                        oacc[oidx][:, ds(c * B_CHUNK, B_CHUNK)],
                        y[:].rearrange("p (b d) -> p b d", d=D),
                        axis=mybir.AxisListType.X,
                        op=mybir.AluOpType.add,
                    )

            def emit_x0b(c):
                # x0 broadcast tile for chunk c (used by its L2/L3 z-builds):
                # stride-0 partition source -> every partition holds x0^T chunk.
                # Split so consumers can start on the first quarter.
                if c >= N_CHUNKS or ("x0b", c) in h_tiles:
                    return
                x0b = x0bpool.tile([128, F0, CHUNK], F16, tag="x0b")
                nq = x0b_split
                w = F0 // nq
                for q in range(nq):
                    eng = nc.sync if q % 2 == 0 else nc.scalar
                    eng.dma_start(
                        out=x0b[:, ds(q * w, w), :],
                        in_=x0t[c, ds(q * w, w), :].partition_broadcast(128))
                h_tiles[("x0b", c)] = x0b

            def emit_l1_mms(c):
                if c >= N_CHUNKS:
                    return
                ps = [pspool.tile([FI, CHUNK], F32, tag="ps", name=f"ps1_{c}_{i}")
                      for i in range(2)]
                z1t = l1pool.tile([128, L1_T, CHUNK], F16, tag="l1z")
                nc.sync.dma_start(out=z1t[:], in_=z1[c])
                # h-half (n=1) first so the h1 activation can fire mid-stream
                for n in (1, 0):
                    for t in range(L1_T):
                        nc.tensor.matmul(ps[n][:],
                                         lhsT=w1s_sb[:, t, ds(n * FI, FI)],
                                         rhs=z1t[:, t, :], start=(t == 0),
                                         stop=(t == L1_T - 1))
                h_tiles[("ps1", c)] = ps

            def emit_h1_act(c):
                # h1 conversion (critical path: next layer's builds wait on it)
                if c >= N_CHUNKS:
                    return
                ps = h_tiles[("ps1", c)]
                h1 = hpool.tile([FI, CHUNK], F16, tag="h1")
                nc.scalar.activation(h1[:], ps[1][:], RELU, bias=bias[("b1", 1)][:])
                h_tiles[("h1", c)] = h1
                act_pool_half(ps[0], bias[("b1", 0)][:], 1.0, 0, c)

            cvt_rr = [0]

            def emit_l23(c, layer):
                w_sb = w2_sb if layer == 2 else w3_sb
                bnm = "b2" if layer == 2 else "b3"
                hin = h_tiles[("h1", c)] if layer == 2 else h_tiles[("h2", c)]
                x0b = h_tiles[("x0b", c)]
                ps = [pspool.tile([FI, CHUNK], F32, tag="ps", name=f"ps{layer}_{c}_{i}")
                      for i in range(2)]
                if layer == 3:
                    flush_reduces()
                z8s = []
                for q in range(NGRP):
                    z16 = z16pool.tile([FI, GG, CHUNK], F16, tag="z16")
                    nc.vector.tensor_mul(
                        z16[:], hin[:].unsqueeze(1).broadcast_to((FI, GG, CHUNK)),
                        x0b[:, ds(q * GG, GG), :])
                    z8 = z8pool.tile([FI, GG, CHUNK], F8, tag="z8")
                    if cvt_rr[0] % 4 < cvt_pool_share:
                        nc.gpsimd.tensor_copy(z8[:], z16[:])
                    else:
                        nc.scalar.activation(z8[:], z16[:], COPY)
                    cvt_rr[0] += 1
                    z8s.append(z8)
                    if layer == 3:
                        # interleave MMs per group (frees z8 tiles quickly)
                        for i in range(GG // 2):
                            p_idx = q * (GG // 2) + i
                            for n in range(2):
                                nc.tensor.matmul(
                                    ps[n][:], lhsT=w_sb[:, p_idx, :, n, :],
                                    rhs=z8[:, ds(2 * i, 2), :],
                                    start=(p_idx == 0), stop=(p_idx == NPAIR - 1),
                                    perf_mode=DR)
                if layer == 2:
                    # h-half (n=1) MMs first across all pairs: the h2
                    # activation fires at the half-way point of the L2 MM
                    # stream, so L3's z-builds overlap L2's pooled-half MMs.
                    for n in (1, 0):
                        for q in range(NGRP):
                            for i in range(GG // 2):
                                p_idx = q * (GG // 2) + i
                                nc.tensor.matmul(
                                    ps[n][:], lhsT=w_sb[:, p_idx, :, n, :],
                                    rhs=z8s[q][:, ds(2 * i, 2), :],
                                    start=(p_idx == 0), stop=(p_idx == NPAIR - 1),
                                    perf_mode=DR)
                            if n == 1 and q == NGRP - 1:
                                h2 = hpool.tile([FI, CHUNK], F16, tag="h2")
                                nc.scalar.activation(h2[:], ps[1][:], RELU,
                                                     bias=bias[(bnm, 1)][:],
                                                     scale=ISCALE)
                                h_tiles[("h2", c)] = h2
                    act_pool_half(ps[0], bias[(bnm, 0)][:], ISCALE, 1, c)
                else:
                    for n in range(2):
                        act_pool_half(ps[n], bias[(bnm, n)][:], ISCALE, 2 + n, c)

            def emit_body():
                emit_x0b(0)
                emit_l1_mms(0)
                emit_h1_act(0)
                if reps == 1:
                    nc.sync.dma_start(out=w2_sb[:], in_=w2[:])
                    nc.scalar.dma_start(out=w3_sb[:], in_=w3[:])
                for c in range(N_CHUNKS):
                    emit_x0b(c + 1)
                    emit_l23(c, 2)
                    emit_l1_mms(c + 1)
                    emit_l23(c, 3)
                    emit_h1_act(c + 1)
                    flush_y_acts()
                flush_reduces()
                for k in range(4):
                    nc.sync.dma_start(out=out[ds(k * FI, FI), :], in_=oacc[k][:])

            if reps == 1:
                emit_body()
            else:
                with tc.For_i(0, reps, 1):
                    emit_body()

    nc.compile()
    return nc


def _get_nc(reps=1, **kw):
    key = (reps, tuple(sorted(kw.items())))
    if key not in _BUILD_CACHE:
        _BUILD_CACHE[key] = _build(reps, **kw)
    return _BUILD_CACHE[key]


def _sym_indices():
    fi, gi = np.triu_indices(F0)          # f <= g, 820 pairs
    return fi, gi


def _prep_inputs(inputs, W1, b1, W2, b2, W3, b3):
    """Host-side shard + layout prep."""
    import ml_dtypes
    f16 = np.float16
    E4 = ml_dtypes.float8_e4m3
    fi, gi = _sym_indices()
    # symmetrized W1: row r=(f,g): W1[f*40+g] + (f<g ? W1[g*40+f] : 0)
    A = np.asarray(W1, np.float32).reshape(F0, F0, NOUT)
    w1sym = A[fi, gi] + np.where((fi < gi)[:, None], A[gi, fi], 0.0)  # [820, 256]
    w1p = np.zeros((L1_T * 128, NOUT), np.float32)
    w1p[:NSYM] = w1sym
    w1h = np.ascontiguousarray(
        w1p.reshape(L1_T, 128, NOUT).transpose(1, 0, 2)).astype(f16)
    # fp8-packed L2/L3 weights, prescaled by 2^WSHIFT
    def pack_w(W):
        Wq = np.asarray(W, np.float32).reshape(FI, F0, NOUT) * WSCALE
        Wq = np.clip(Wq, -239.0, 239.0)
        return np.ascontiguousarray(
            Wq.reshape(FI, NPAIR, 2, 2, 128)).astype(E4)
    w2h, w3h = pack_w(W2), pack_w(W3)
    b1f = np.ascontiguousarray(b1, dtype=np.float32)
    b2f = np.ascontiguousarray(b2, dtype=np.float32)
    b3f = np.ascontiguousarray(b3, dtype=np.float32)
    in_maps = []
    for core in range(N_CORES):
        xc = np.asarray(inputs[core * B_CORE:(core + 1) * B_CORE], np.float32)
        t = xc.transpose(1, 0, 2).reshape(F0, BD)                # [40, 2048]
        tc4 = np.ascontiguousarray(
            t.reshape(F0, N_CHUNKS, CHUNK).transpose(1, 0, 2))   # [4, 40, 512] f32
        x0tc = tc4.astype(f16)
        # symmetric z1 rows from the fp16-rounded x0 (matches device x0 path)
        tf = x0tc.astype(np.float32)
        z1s = np.zeros((N_CHUNKS, L1_T * 128, CHUNK), np.float32)
        z1s[:, :NSYM] = tf[:, fi, :] * tf[:, gi, :]
        z1c = np.ascontiguousarray(
            z1s.reshape(N_CHUNKS, L1_T, 128, CHUNK).transpose(0, 2, 1, 3)
        ).astype(f16)
        in_maps.append({
            "x0t": x0tc, "z1": z1c,
            "w1s": w1h, "w2": w2h, "w3": w3h,
            "b1": b1f, "b2": b2f, "b3": b3f,
        })
    return in_maps


def _unshard(results):
    # per-core out: [512 fields, 64 local batch] -> [B, 512]
    full = np.concatenate([r["out"] for r in results], axis=1)   # [512, 512]
    return np.ascontiguousarray(full.T)


def kernel(inputs, W1, b1, W2, b2, W3, b3):
    from concourse.bass_utils import run_bass_kernel_spmd
    inputs, W1, W2, W3 = (np.asarray(t, dtype=np.float32)
                          for t in (inputs, W1, W2, W3))
    b1, b2, b3 = (np.asarray(t, dtype=np.float32) for t in (b1, b2, b3))
    nc = _get_nc(reps=1)
    in_maps = _prep_inputs(inputs, W1, b1, W2, b2, W3, b3)
    res = run_bass_kernel_spmd(nc, in_maps, list(range(N_CORES)))
    return _unshard(res.results)


# revision 9
# speedup vs baseline: 2.0283x; 2.0283x over previous
"""CIN (Compressed Interaction Network) forward pass on 8 Trainium2 cores.

Reference computation (per sample b, per embedding dim d — fully pointwise
in (b, d)):
    x0 = inputs[b, :, d]                    # [40]
    h  = x0
    per layer i (W_i maps Fi*40 -> 256):
        z  = outer(h, x0).flatten()         # [Fi * 40], index f*40+g
        y  = relu(z @ W_i + b_i)            # [256]
        outputs_i = y[:128] (i<3) else y    # pooled
        h  = y[128:]                        # fields fed forward
    out[b] = sum_d concat(outputs)          # [512]

Sharding: data-parallel over batch (64 samples/core), weights replicated.

Per-core kernel strategy (v3 — measured-rate balanced):
  - Blocked over bd = (b*32 + d) in chunks of 512 columns; column-major
    layouts (h^T [fields, bd]).
  - Layer 1 exploits z-symmetry (z[f,g]=z[g,f]): contraction shrinks from
    1600 to 820 rows (padded to 1024) with symmetrized weights
    W1s[(f,g)] = W1[f,g]+W1[g,f], and runs in fp8-e4m3 DoubleRow mode
    (4 k-tiles of 256 rows).  z1 tiles are pure functions of the input —
    precomputed on the host directly in fp8 and streamed from HBM, so L1
    needs no on-device z work at all.
  - Layer 2 runs bf16-style fp16 matmuls straight off the DVE-built z16
    tiles (no conversion cost); layer 3 runs fp8 DoubleRow (each matmul
    contracts 256 rows) with z16->fp8 conversion on the Scalar engine
    (the GpSimd engine takes a small fixed share — measured ~4x slower
    per element, so it only gets ~1 in 5 groups).
  - z tiles are built on the Vector engine in fp16 2x-packed mode, groups
    of 8 g-values: z16[f, j, :] = h[f, :] * x0b[f, g0+j, :].
  - The x0 row needed on all 128 partitions is materialized once per chunk
    by DMAs with a stride-0 partition source (quartered, alternating HWDGE
    rings so consumers start on the first quarter).
  - L2's matmul stream runs its h-half (n=1) first so the h2 activation
    fires at the stream's midpoint and L3's z-builds overlap L2's
    pooled-half matmuls; L1's h-half runs first for the same reason.
  - Scalar engine applies scale+bias+ReLU straight out of PSUM -> fp16;
    weights are host-prescaled by 2^s (undone by the activation scale) to
    center them in fp8 range.
  - Vector engine reduces y-halves over d (groups of 32) into the output
    accumulators, deferred a chunk so the in-order DVE stream never
    stalls; final DMA writes [512 fields, 64 batch] per core.
  - Layer order is software-pipelined L1(0), [L2(c), L1(c+1), L3(c)] so the
    tensor engine never waits on the h hand-off between layers.
"""

import numpy as np

import concourse.bass as bass
import concourse.tile as tile
from concourse import bacc, mybir
from concourse.bass import ds

F32 = mybir.dt.float32
F16 = mybir.dt.float16
F8 = mybir.dt.float8e4

B, F0, D = 512, 40, 32
N_CORES = 8
B_CORE = B // N_CORES            # 64
BD = B_CORE * D                  # 2048
CHUNK = 512
N_CHUNKS = BD // CHUNK           # 4
B_CHUNK = CHUNK // D             # 16 batch rows per chunk
FI = 128                         # h fields for layers 2/3
NOUT = 256
NSYM = F0 * (F0 + 1) // 2        # 820
L1_T = 4                         # 4 DoubleRow k-tiles x 256 = 1024 >= 820
NPAIR = F0 // 2                  # 20 DoubleRow pairs per layer
GG = 8                           # g-values per DVE z-build op (4 pairs)
NGRP = F0 // GG                  # 5 groups per layer-chunk
W1SHIFT = 10                     # W1s prescale exponent (fp8 range centering)
W23SHIFT = 11                    # W2/W3 prescale exponent
I1SCALE = float(2.0 ** -W1SHIFT)
I23SCALE = float(2.0 ** -W23SHIFT)
RELU = mybir.ActivationFunctionType.Relu
COPY = mybir.ActivationFunctionType.Copy
DR = mybir.MatmulPerfMode.DoubleRow

_BUILD_CACHE = {}


def _build(reps=1, trace_sim=False, psum_bufs=8, z8_bufs=4, z16_bufs=4,
           x0b_split=4, pool_cvt_every=5, l2_dtype="f16", l3_dtype="f8"):
    """Build + schedule + bacc-compile the per-core program."""
    nc = bacc.Bacc("TRN2", target_bir_lowering=False, debug=False,
                   num_devices=N_CORES)

    x0t = nc.declare_dram_parameter("x0t", [N_CHUNKS, F0, CHUNK], F16, isOutput=False)
    z1 = nc.declare_dram_parameter("z1", [N_CHUNKS, 128, L1_T, 2, CHUNK], F8, isOutput=False)
    w1s = nc.declare_dram_parameter("w1s", [128, L1_T, 2, 2, 128], F8, isOutput=False)
    w2 = nc.declare_dram_parameter(
        "w2", [FI, NPAIR, 2, 2, 128] if l2_dtype == "f8" else [FI, F0, NOUT],
        F8 if l2_dtype == "f8" else F16, isOutput=False)
    w3 = nc.declare_dram_parameter(
        "w3", [FI, NPAIR, 2, 2, 128] if l3_dtype == "f8" else [FI, F0, NOUT],
        F8 if l3_dtype == "f8" else F16, isOutput=False)
    b1 = nc.declare_dram_parameter("b1", [NOUT], F32, isOutput=False)
    b2 = nc.declare_dram_parameter("b2", [NOUT], F32, isOutput=False)
    b3 = nc.declare_dram_parameter("b3", [NOUT], F32, isOutput=False)
    out = nc.declare_dram_parameter("out", [4 * FI, B_CORE], F32, isOutput=True)

    with tile.TileContext(nc, trace_sim=trace_sim) as tc:
        import contextlib
        with contextlib.ExitStack() as ctx:
            wpool = ctx.enter_context(tc.tile_pool(name="w", bufs=1))
            opool = ctx.enter_context(tc.tile_pool(name="o", bufs=1))
            x0bpool = ctx.enter_context(tc.tile_pool(name="x0b", bufs=2))
            l1pool = ctx.enter_context(tc.tile_pool(name="l1", bufs=2))
            z16pool = ctx.enter_context(tc.tile_pool(name="z16", bufs=z16_bufs))
            z8pool = ctx.enter_context(tc.tile_pool(name="z8", bufs=z8_bufs))
            hpool = ctx.enter_context(tc.tile_pool(name="h", bufs=3))
            ypool = ctx.enter_context(tc.tile_pool(name="y", bufs=6))
            pspool = ctx.enter_context(tc.tile_pool(name="ps", bufs=psum_bufs, space="PSUM"))

            # ---- resident constants ----
            w1s_sb = wpool.tile([128, L1_T, 2, 2, 128], F8, tag="w1s")
            nc.scalar.dma_start(out=w1s_sb[:], in_=w1s[:])
            w2_sb = wpool.tile(list(w2.shape), w2.dtype, tag="w2")
            w3_sb = wpool.tile(list(w3.shape), w3.dtype, tag="w3")
            if reps != 1:
                nc.sync.dma_start(out=w2_sb[:], in_=w2[:])
                nc.scalar.dma_start(out=w3_sb[:], in_=w3[:])
            bias = {}
            for nm, t in (("b1", b1), ("b2", b2), ("b3", b3)):
                bt = wpool.tile([128, 2], F32, tag=f"bias_{nm}", name=f"bias_{nm}")
                nc.scalar.dma_start(out=bt[:],
                                    in_=t[:].rearrange("(n p) -> p n", p=128))
                for half in range(2):
                    bias[(nm, half)] = bt[:, half:half + 1]
            # output accumulators [fields(128) x batch], one per field block
            oacc = [opool.tile([FI, B_CORE], F32, tag=f"oacc{k}", name=f"oacc{k}")
                    for k in range(4)]

            pending_acts = []     # deferred (psum, bias_ap, scale, oidx, c)
            pending_reduces = []  # deferred (y, oidx, c)
            h_tiles = {}

            def act_pool_half(ps_half, bias_ap, sc, oidx, c):
                pending_acts.append((ps_half, bias_ap, sc, oidx, c))

            def flush_y_acts():
                while pending_acts:
                    ps_half, bias_ap, sc, oidx, c = pending_acts.pop(0)
                    y = ypool.tile([FI, CHUNK], F16, tag="y", name=f"y_{oidx}_{c}")
                    nc.scalar.activation(y[:], ps_half[:], RELU, bias=bias_ap,
                                         scale=sc)
                    pending_reduces.append((y, oidx, c))

            def flush_reduces():
                # d-sum reduces on the DVE; emitted a chunk later than their
                # y so the in-order DVE stream never stalls waiting on ACT.
                while pending_reduces:
                    y, oidx, c = pending_reduces.pop(0)
                    nc.vector.tensor_reduce(
                        oacc[oidx][:, ds(c * B_CHUNK, B_CHUNK)],
                        y[:].rearrange("p (b d) -> p b d", d=D),
                        axis=mybir.AxisListType.X,
                        op=mybir.AluOpType.add,
                    )

            def emit_x0b(c):
                if c >= N_CHUNKS or ("x0b", c) in h_tiles:
                    return
                x0b = x0bpool.tile([128, F0, CHUNK], F16, tag="x0b")
                nq = x0b_split
                w = F0 // nq
                for q in range(nq):
                    eng = nc.sync if q % 2 == 0 else nc.scalar
                    eng.dma_start(
                        out=x0b[:, ds(q * w, w), :],
                        in_=x0t[c, ds(q * w, w), :].partition_broadcast(128))
                h_tiles[("x0b", c)] = x0b

            def emit_l1_mms(c):
                if c >= N_CHUNKS:
                    return
                ps = [pspool.tile([FI, CHUNK], F32, tag="ps", name=f"ps1_{c}_{i}")
                      for i in range(2)]
                z1t = l1pool.tile([128, L1_T, 2, CHUNK], F8, tag="l1z")
                nc.sync.dma_start(out=z1t[:], in_=z1[c])
                # h-half (n=1) first so the h1 activation can fire mid-stream
                for n in (1, 0):
                    for t in range(L1_T):
                        nc.tensor.matmul(ps[n][:],
                                         lhsT=w1s_sb[:, t, :, n, :],
                                         rhs=z1t[:, t, :, :], start=(t == 0),
                                         stop=(t == L1_T - 1), perf_mode=DR)
                h_tiles[("ps1", c)] = ps

            def emit_h1_act(c):
                if c >= N_CHUNKS:
                    return
                ps = h_tiles[("ps1", c)]
                h1 = hpool.tile([FI, CHUNK], F16, tag="h1")
                nc.scalar.activation(h1[:], ps[1][:], RELU,
                                     bias=bias[("b1", 1)], scale=I1SCALE)
                h_tiles[("h1", c)] = h1
                act_pool_half(ps[0], bias[("b1", 0)], I1SCALE, 0, c)

            cvt_rr = [0]

            def emit_l23(c, layer):
                w_sb = w2_sb if layer == 2 else w3_sb
                dt8 = (l2_dtype if layer == 2 else l3_dtype) == "f8"
                bnm = "b2" if layer == 2 else "b3"
                hin = h_tiles[("h1", c)] if layer == 2 else h_tiles[("h2", c)]
                x0b = h_tiles[("x0b", c)]
                ps = [pspool.tile([FI, CHUNK], F32, tag="ps", name=f"ps{layer}_{c}_{i}")
                      for i in range(2)]
                if layer == 3:
                    flush_reduces()
                zs = []
                for q in range(NGRP):
                    z16 = z16pool.tile([FI, GG, CHUNK], F16, tag="z16")
                    nc.vector.tensor_mul(
                        z16[:], hin[:].unsqueeze(1).broadcast_to((FI, GG, CHUNK)),
                        x0b[:, ds(q * GG, GG), :])
                    if dt8:
                        z8 = z8pool.tile([FI, GG, CHUNK], F8, tag="z8")
                        cvt_rr[0] += 1
                        if cvt_rr[0] % pool_cvt_every == 0:
                            nc.gpsimd.tensor_copy(z8[:], z16[:])
                        else:
                            nc.scalar.activation(z8[:], z16[:], COPY)
                        zs.append(z8)
                    else:
                        zs.append(z16)
                    if layer == 3:
                        emit_mm_group(ps, w_sb, dt8, zs[q], q,
                                      start_q=0, stop_q=NGRP - 1)
                if layer == 2:
                    # h-half (n=1) MMs first across all groups: the h2
                    # activation fires at the stream midpoint, so L3's
                    # z-builds overlap L2's pooled-half MMs.
                    for n in (1, 0):
                        for q in range(NGRP):
                            emit_mm_half(ps[n], w_sb, dt8, zs[q], q, n,
                                         start_q=0, stop_q=NGRP - 1)
                        if n == 1:
                            h2 = hpool.tile([FI, CHUNK], F16, tag="h2")
                            nc.scalar.activation(h2[:], ps[1][:], RELU,
                                                 bias=bias[(bnm, 1)],
                                                 scale=I23SCALE if dt8 else 1.0)
                            h_tiles[("h2", c)] = h2
                    act_pool_half(ps[0], bias[(bnm, 0)],
                                  I23SCALE if dt8 else 1.0, 1, c)
                else:
                    for n in range(2):
                        act_pool_half(ps[n], bias[(bnm, n)],
                                      I23SCALE if dt8 else 1.0, 2 + n, c)

            def emit_mm_half(psn, w_sb, dt8, z, q, n, start_q, stop_q):
                if dt8:
                    for i in range(GG // 2):
                        p_idx = q * (GG // 2) + i
                        nc.tensor.matmul(
                            psn[:], lhsT=w_sb[:, p_idx, :, n, :],
                            rhs=z[:, ds(2 * i, 2), :],
                            start=(q == start_q and i == 0),
                            stop=(q == stop_q and i == GG // 2 - 1),
                            perf_mode=DR)
                else:
                    for j in range(GG):
                        g = q * GG + j
                        nc.tensor.matmul(
                            psn[:], lhsT=w_sb[:, g, ds(n * FI, FI)],
                            rhs=z[:, j, :],
                            start=(q == start_q and j == 0),
                            stop=(q == stop_q and j == GG - 1))

            def emit_mm_group(ps, w_sb, dt8, z, q, start_q, stop_q):
                if dt8:
                    for i in range(GG // 2):
                        p_idx = q * (GG // 2) + i
                        for n in range(2):
                            nc.tensor.matmul(
                                ps[n][:], lhsT=w_sb[:, p_idx, :, n, :],
                                rhs=z[:, ds(2 * i, 2), :],
                                start=(q == start_q and i == 0),
                                stop=(q == stop_q and i == GG // 2 - 1),
                                perf_mode=DR)
                else:
                    for j in range(GG):
                        g = q * GG + j
                        for n in range(2):
                            nc.tensor.matmul(
                                ps[n][:], lhsT=w_sb[:, g, ds(n * FI, FI)],
                                rhs=z[:, j, :],
                                start=(q == start_q and j == 0),
                                stop=(q == stop_q and j == GG - 1))

            def emit_body():
                emit_x0b(0)
                emit_l1_mms(0)
                emit_h1_act(0)
                if reps == 1:
                    nc.sync.dma_start(out=w2_sb[:], in_=w2[:])
                    nc.scalar.dma_start(out=w3_sb[:], in_=w3[:])
                for c in range(N_CHUNKS):
                    emit_x0b(c + 1)
                    emit_l23(c, 2)
                    emit_l1_mms(c + 1)
                    emit_l23(c, 3)
                    emit_h1_act(c + 1)
                    flush_y_acts()
                flush_reduces()
                for k in range(4):
                    nc.sync.dma_start(out=out[ds(k * FI, FI), :], in_=oacc[k][:])

            if reps == 1:
                emit_body()
            else:
                with tc.For_i(0, reps, 1):
                    emit_body()

    nc.compile()
    return nc


def _get_nc(reps=1, **kw):
    key = (reps, tuple(sorted(kw.items())))
    if key not in _BUILD_CACHE:
        _BUILD_CACHE[key] = _build(reps, **kw)
    return _BUILD_CACHE[key]


def _sym_indices():
    fi, gi = np.triu_indices(F0)          # f <= g, 820 pairs
    return fi, gi


def _prep_inputs(inputs, W1, b1, W2, b2, W3, b3,
                 l2_dtype="f16", l3_dtype="f8"):
    """Host-side shard + layout prep."""
    import ml_dtypes
    f16 = np.float16
    E4 = ml_dtypes.float8_e4m3
    fi, gi = _sym_indices()
    # symmetrized W1: row r=(f,g): W1[f*40+g] + (f<g ? W1[g*40+f] : 0)
    A = np.asarray(W1, np.float32).reshape(F0, F0, NOUT)
    w1sym = A[fi, gi] + np.where((fi < gi)[:, None], A[gi, fi], 0.0)  # [820, 256]
    w1p = np.zeros((L1_T * 256, NOUT), np.float32)
    w1p[:NSYM] = w1sym * float(2.0 ** W1SHIFT)
    w1p = np.clip(w1p, -239.0, 239.0)
    # [r, o] -> [p, t, i, n, m] with r = t*256 + i*128 + p, o = n*128 + m
    w1h = np.ascontiguousarray(
        w1p.reshape(L1_T, 2, 128, 2, 128).transpose(2, 0, 1, 3, 4)).astype(E4)

    def pack_w(W, mode):
        Wf = np.asarray(W, np.float32).reshape(FI, F0, NOUT)
        if mode == "f16":
            return np.ascontiguousarray(Wf).astype(f16)
        Wq = np.clip(Wf * float(2.0 ** W23SHIFT), -239.0, 239.0)
        return np.ascontiguousarray(
            Wq.reshape(FI, NPAIR, 2, 2, 128)).astype(E4)
    w2h, w3h = pack_w(W2, l2_dtype), pack_w(W3, l3_dtype)
    b1f = np.ascontiguousarray(b1, dtype=np.float32)
    b2f = np.ascontiguousarray(b2, dtype=np.float32)
    b3f = np.ascontiguousarray(b3, dtype=np.float32)
    in_maps = []
    for core in range(N_CORES):
        xc = np.asarray(inputs[core * B_CORE:(core + 1) * B_CORE], np.float32)
        t = xc.transpose(1, 0, 2).reshape(F0, BD)                # [40, 2048]
        tc4 = np.ascontiguousarray(
            t.reshape(F0, N_CHUNKS, CHUNK).transpose(1, 0, 2))   # [4, 40, 512] f32
        x0tc = tc4.astype(f16)
        # symmetric z1 rows from the fp16-rounded x0 (matches device x0 path)
        tf = x0tc.astype(np.float32)
        z1s = np.zeros((N_CHUNKS, L1_T * 256, CHUNK), np.float32)
        z1s[:, :NSYM] = tf[:, fi, :] * tf[:, gi, :]
        # [c, r, x] -> [c, p, t, i, x] with r = t*256 + i*128 + p
        z1c = np.ascontiguousarray(
            z1s.reshape(N_CHUNKS, L1_T, 2, 128, CHUNK).transpose(0, 3, 1, 2, 4)
        ).astype(E4)
        in_maps.append({
            "x0t": x0tc, "z1": z1c,
            "w1s": w1h, "w2": w2h, "w3": w3h,
            "b1": b1f, "b2": b2f, "b3": b3f,
        })
    return in_maps


def _unshard(results):
    # per-core out: [512 fields, 64 local batch] -> [B, 512]
    full = np.concatenate([r["out"] for r in results], axis=1)   # [512, 512]
    return np.ascontiguousarray(full.T)


def kernel(inputs, W1, b1, W2, b2, W3, b3):
    from concourse.bass_utils import run_bass_kernel_spmd
    inputs, W1, W2, W3 = (np.asarray(t, dtype=np.float32)
                          for t in (inputs, W1, W2, W3))
    b1, b2, b3 = (np.asarray(t, dtype=np.float32) for t in (b1, b2, b3))
    nc = _get_nc(reps=1)
    in_maps = _prep_inputs(inputs, W1, b1, W2, b2, W3, b3)
    res = run_bass_kernel_spmd(nc, in_maps, list(range(N_CORES)))
    return _unshard(res.results)
